# revision 1
# baseline (speedup 1.0000x reference)
"""Multi-head causal attention (B=2, T=2048, E=1024, H=16, D=64) on 8 trn2 cores.

Sharding: core c -> batch b = c // 4, head-group hg = c % 4 (4 heads each).
Per-core: QKV projections for its 4 heads, causal flash attention in
transposed-score layout (S^T[k,q]; softmax denominator folded into a
ones-augmented V matmul), row-parallel output projection producing a partial
[T, E] output. Host sums the 4 partials per batch and adds the bias.

v2: bf16 operands end-to-end, projection/output-projection matmuls paced as
fillers into the attention stream (the exp on the ACT engine is the local
attention bottleneck; interleaved proj matmuls keep the PE busy while exps
drain), contiguous diagonal-block packing (no garbage exp columns), batched
x DMAs, bf16 partial outputs (summed in fp32 on host).
"""
import collections
import sys
from contextlib import ExitStack

sys.path.insert(0, "/opt/trn_rl_repo")

import ml_dtypes
import numpy as np

import concourse.bass as bass
import concourse.tile as tile
from concourse import bacc, mybir
from concourse.bass_utils import run_bass_kernel_spmd

F32 = mybir.dt.float32
BF16 = mybir.dt.bfloat16
FP8 = mybir.dt.float8e4
DR = mybir.MatmulPerfMode.DoubleRow
EXP = mybir.ActivationFunctionType.Exp

WSCALE = 32.0           # host prescale on Wq/Wk/Wv for fp8 range; q,k,v come
                        # out x32, folded into the exp scale and into Wp

B, T, E, H = 2, 2048, 1024, 16
D = E // H              # 64
N_CORES = 8
GH = 4                  # heads per core
GE = GH * D             # 256 per-core projection width
SCALE = float(D) ** -0.5

TCH = 512               # projection t-chunk
NTCH = T // TCH         # 4
KC = 8                  # contraction chunks of 128 over E
QB = 512                # attention q-block
NQB = T // QB           # 4
KB = 128                # attention k-block

PE_NS = 1e9 / 2.4e9     # per moving-free column (bf16)
ACT_NS = 1e9 / 1.2e9    # per free column
EXP_OVH = 217.0         # ACT per-instruction overhead (access + decode)

DEFAULT_OPTS = dict(
    s_bufs=2,
    o_bufs=3,
    p_bufs=6,
    x_bufs=4,
    l_bufs=6,
    on_bufs=8,
    yst_bufs=4,
    norm_splits=1,       # normalize split count (qb < last)
    norm_splits_last=2,  # normalize split count for the last q-block
    sem_lat=400.0,       # pacing fudge: SS-end -> exp-start latency
    ret_lat=1000.0,      # pacing fudge: exp-end -> O-start latency
    lead=0.0,            # pacing margin (ns)
    y_pool_dma=False,    # issue y DMAs from the Pool queue
    y_pool_copy=False,   # gpsimd cannot read PSUM (BIR verifier rejects it)
    end_fill=1200.0,     # filler ns pulled at each stream end (norm window)
    copy_cd=1100.0,      # ns between a proj drain copy and the next slot alloc
    y_defer=1,           # 1: Y(qb) paced into phase qb+1; 3: all saved for last phase
    qk_copy_eng="scalar",
)


def build_program(opts=None):
    o = dict(DEFAULT_OPTS)
    if opts:
        o.update(opts)
    nc = bacc.Bacc("TRN2", target_bir_lowering=False, debug=False, num_devices=N_CORES)

    xh_d = nc.dram_tensor("xh", [E, T], FP8, kind="ExternalInput").ap()
    xl_d = nc.dram_tensor("xl", [E, T], FP8, kind="ExternalInput").ap()
    wqh_d = nc.dram_tensor("wqh", [E, GE], FP8, kind="ExternalInput").ap()
    wql_d = nc.dram_tensor("wql", [E, GE], FP8, kind="ExternalInput").ap()
    wkh_d = nc.dram_tensor("wkh", [E, GE], FP8, kind="ExternalInput").ap()
    wkl_d = nc.dram_tensor("wkl", [E, GE], FP8, kind="ExternalInput").ap()
    wvh_d = nc.dram_tensor("wvh", [E, GE], FP8, kind="ExternalInput").ap()
    wvl_d = nc.dram_tensor("wvl", [E, GE], FP8, kind="ExternalInput").ap()
    wpt_d = nc.dram_tensor("wpt", [GE, E], BF16, kind="ExternalInput").ap()
    tri_d = nc.dram_tensor("tri", [KB, KB], BF16, kind="ExternalInput").ap()
    ones_d = nc.dram_tensor("ones", [128, (T // KB) * GH], BF16, kind="ExternalInput").ap()
    y_d = nc.dram_tensor("y", [T, E], BF16, kind="ExternalOutput").ap()

    with tile.TileContext(nc) as tc:
        with tc.tile_pool(name="weights", bufs=1) as wpool, \
             tc.tile_pool(name="qk", bufs=1) as qkpool, \
             tc.tile_pool(name="vsb", bufs=1) as vpool, \
             tc.tile_pool(name="xin", bufs=o["x_bufs"]) as xpool, \
             tc.tile_pool(name="ptile", bufs=o["p_bufs"]) as ppool, \
             tc.tile_pool(name="lbc", bufs=o["l_bufs"]) as lpool, \
             tc.tile_pool(name="onorm", bufs=o["on_bufs"]) as onpool, \
             tc.tile_pool(name="ystage", bufs=o["yst_bufs"]) as ypool, \
             tc.tile_pool(name="s_ps", bufs=o["s_bufs"], space="PSUM") as s_ps, \
             tc.tile_pool(name="pv_ps", bufs=1, space="PSUM") as pv_ps, \
             tc.tile_pool(name="o_ps", bufs=o["o_bufs"], space="PSUM") as o_ps:
            qk_ps = v_ps = pv_ps

            KC2 = KC // 2
            wq_sb = [wpool.tile([128, KC2, 2, GE], FP8, name=f"wq{i}") for i in range(2)]
            wk_sb = [wpool.tile([128, KC2, 2, GE], FP8, name=f"wk{i}") for i in range(2)]
            wv_sb = [wpool.tile([128, KC2, 2, GE], FP8, name=f"wv{i}") for i in range(2)]
            wp_sb = wpool.tile([128, 2, E], BF16)
            tri_sb = wpool.tile([KB, KB], BF16)
            ones_sb = wpool.tile([128, (T // KB) * GH], BF16)

            qt_sb = qkpool.tile([128, 2, T], BF16)   # pair-stacked Q^T
            kt_sb = qkpool.tile([128, 2, T], BF16)   # pair-stacked K^T
            v_sb = vpool.tile([128, T // KB, GH * (D + 1)], BF16)
            v_ones = v_sb.rearrange("p b (h c) -> p (b h) c", c=D + 1)[:, :, D:D + 1]

            xts = [None] * NTCH  # per-tch ([hi, lo]) [128, KC2, 2, TCH] tiles

            def dr(ap3):
                # dram [rows, n] -> DoubleRow sbuf layout [p, c2, i, n]
                return ap3.rearrange("(c i p) n -> p c i n", i=2, p=128)

            def w_c2(w_sb, w_d, c2a, c2b):
                for hl in range(2):
                    nc.sync.dma_start(
                        out=w_sb[hl][:, c2a:c2b, :, :],
                        in_=dr(w_d[hl][c2a * 256:c2b * 256, :]))

            def emit_x_dma(tch):
                xts[tch] = [xpool.tile([128, KC2, 2, TCH], FP8, tag="xt",
                                       name=f"x{tch}_{hl}") for hl in range(2)]
                xsrc = [x_d[:, tch * TCH:(tch + 1) * TCH] for x_d in (xh_d, xl_d)]
                if tch == 0:
                    # prologue: every DMA ordered by first use (copies are a
                    # serial resource); c2-chunk pieces so compute starts early
                    for c2 in range(KC2):
                        for hl in range(2):
                            nc.sync.dma_start(out=xts[0][hl][:, c2, :, :],
                                              in_=dr(xsrc[hl])[:, c2, :, :])
                        w_c2(wq_sb, (wqh_d, wql_d), c2, c2 + 1)
                        w_c2(wk_sb, (wkh_d, wkl_d), c2, c2 + 1)
                    w_c2(wv_sb, (wvh_d, wvl_d), 0, 2)
                    nc.sync.dma_start(out=tri_sb[:], in_=tri_d)
                    nc.sync.dma_start(out=ones_sb[:], in_=ones_d)
                    w_c2(wv_sb, (wvh_d, wvl_d), 2, 4)
                    emit_x_dma(1)
                    nc.sync.dma_start(
                        out=wp_sb[:], in_=wpt_d.rearrange("(c p) n -> p c n", p=128))
                else:
                    for hl in range(2):
                        nc.sync.dma_start(out=xts[tch][hl][:], in_=dr(xsrc[hl]))

            # ---- pacing state ------------------------------------------------
            clk = {"pe": 0.0, "act": 0.0, "last_copy": -1e9}
            fillers = collections.deque()  # (tag, pe_ns, emit_fn)

            def mm(pe_ns):
                clk["pe"] += pe_ns

            def emit_one():
                tag, pe_ns, fn, _alloc = fillers.popleft()
                marks = fn() or ()
                clk["pe"] += pe_ns
                if "copy" in marks:
                    clk["last_copy"] = clk["pe"]

            def pace(target):
                # hold back a unit that re-allocates the shared proj psum slot
                # until the previous group's drain copy had time to run
                while fillers and clk["pe"] < target - o["lead"]:
                    if fillers[0][3] and clk["pe"] < clk["last_copy"] + o["copy_cd"]:
                        break
                    emit_one()

            def drain(tag_pred):
                while any(tag_pred(t) for t, _, _, _ in fillers):
                    emit_one()

            # ---- projection units -------------------------------------------
            TERMS = ((0, 0), (1, 0), (0, 1))  # (w hi/lo, x hi/lo): hh, lh, hl

            def make_qk_units(tch, w_sb, dst, knm, qp_h):
                def qk_u(pair, key, c2):
                    def fn():
                        if c2 == 0:
                            qp_h[key] = qk_ps.tile([128, TCH], F32, tag="pv",
                                                   name=f"qk_{tch}_{key}")
                        for ti, (wl, xl) in enumerate(TERMS):
                            nc.tensor.matmul(
                                qp_h[key][:],
                                w_sb[wl][:, c2, :, pair * 128:(pair + 1) * 128],
                                xts[tch][xl][:, c2, :, :],
                                start=(c2 == 0 and ti == 0),
                                stop=(c2 == KC2 - 1 and ti == len(TERMS) - 1),
                                perf_mode=DR)
                        if c2 == KC2 - 1:
                            # ACT-side copy: the DVE queue is congested with
                            # norms/V/Y work and these copies gate phase starts
                            if o["qk_copy_eng"] == "scalar":
                                nc.scalar.copy(out=dst[:, pair, tch * TCH:(tch + 1) * TCH],
                                               in_=qp_h[key][:])
                            else:
                                nc.vector.tensor_copy(out=dst[:, pair, tch * TCH:(tch + 1) * TCH],
                                                      in_=qp_h[key][:])
                            return ("copy",)
                    return fn
                return [(f"{knm}p{pair}", 3 * (TCH // 2) * PE_NS,
                         qk_u(pair, f"{knm}{pair}", c2), c2 == 0)
                        for pair in range(2) for c2 in range(KC2)]

            def q_units(tch):
                return make_qk_units(tch, wq_sb, qt_sb, "q", {})

            def kv_units(tch):
                units = make_qk_units(tch, wk_sb, kt_sb, "k", {})
                vp_h = {}

                def v_u(tsub, half):
                    def fn():
                        if half == 0:
                            vp_h[tsub] = v_ps.tile([128, GE], F32, tag="pv",
                                                   name=f"vp{tch}_{tsub}")
                        for c2 in range(2 * half, 2 * half + 2):
                            for ti, (wl, xl) in enumerate(TERMS):
                                nc.tensor.matmul(
                                    vp_h[tsub][:],
                                    xts[tch][xl][:, c2, :, tsub * KB:(tsub + 1) * KB],
                                    wv_sb[wl][:, c2, :, :],
                                    start=(c2 == 0 and ti == 0),
                                    stop=(c2 == KC2 - 1 and ti == len(TERMS) - 1),
                                    perf_mode=DR)
                        if half == 1:
                            tb = tch * (TCH // KB) + tsub
                            nc.vector.tensor_copy(
                                out=v_sb[:, tb, :].rearrange("p (h c) -> p h c", c=D + 1)[:, :, 0:D],
                                in_=vp_h[tsub].rearrange("p (h c) -> p h c", c=D))
                            return ("copy",)
                    return fn

                # single shared psum slot: groups must stay contiguous
                k0 = [u for u in units if u[0] == "kp0"]
                k1 = [u for u in units if u[0] == "kp1"]
                vs = [("v", 3 * GE * PE_NS, v_u(t, half), half == 0)
                      for t in range(TCH // KB) for half in range(2)]
                return k0 + vs + k1

            # ---- output-projection units ------------------------------------
            def y_units(qb, onorms):
                q0 = qb * QB
                units = []
                yt_h = {}
                late = qb >= NQB - 2  # runs in phase 3 / tail: pv pool is idle

                def y_unit(qt, nh):
                    def fn():
                        if nh == 0:
                            yt_h[qt] = ypool.tile([128, E], BF16, tag="yt", name=f"yt{qt}")
                        if late:
                            yp = pv_ps.tile([128, 512], F32, tag="pv", name="yp")
                        else:
                            yp = s_ps.tile([128, 512], F32, tag="s", name="yp")
                        for pair in range(2):
                            nc.tensor.matmul(yp[:],
                                             onorms[pair][:, qt * 128:(qt + 1) * 128],
                                             wp_sb[:, pair, nh * 512:(nh + 1) * 512],
                                             start=(pair == 0), stop=(pair == 1))
                        nc.vector.tensor_copy(out=yt_h[qt][:, nh * 512:(nh + 1) * 512], in_=yp[:])
                        nc.sync.dma_start(
                            out=y_d[q0 + qt * 128:q0 + (qt + 1) * 128, nh * 512:(nh + 1) * 512],
                            in_=yt_h[qt][:, nh * 512:(nh + 1) * 512])
                    return fn

                for qt in range(QB // 128):
                    for nh in range(2):
                        units.append(("y", 2 * 512 * PE_NS, y_unit(qt, nh), False))
                return units

            # ---- attention stream -------------------------------------------
            def slot(hb):
                return slice(hb * (D + 1), (hb + 1) * (D + 1))

            def normalize(o_p, onorm, h, splits=1):
                w = QB // splits
                for s in range(splits):
                    qs = slice(s * w, (s + 1) * w)
                    strip = lpool.tile([1, w], F32, tag="strip")
                    nc.vector.reciprocal(out=strip[:], in_=o_p[D:D + 1, qs])
                    lb = lpool.tile([D, w], F32, tag="lb")
                    nc.gpsimd.partition_broadcast(lb[:], strip[:])
                    nc.vector.tensor_mul(onorm[h * D:(h + 1) * D, qs], o_p[0:D, qs], lb[:])

            def stream(qb, pair, h, onorm, splits, prediag=None):
                q0 = qb * QB
                nk = (q0 + QB) // KB
                nfull = nk - 4
                bsl = slice(h * D, h * D + D)
                hb = pair * 2 + h
                o_p = o_ps.tile([D + 1, QB], F32, tag="o")
                qrhs = qt_sb[bsl, pair, q0:q0 + QB]

                def grp(j2, diag):
                    r0 = (j2 - nfull) * KB if diag else 0
                    r1 = r0 + KB
                    w1 = QB - r1

                    def ss():
                        sp = s_ps.tile([128, 2 * QB], F32, tag="s", name="sp")
                        pt = ppool.tile([128, 2 * QB], BF16, tag="p", name="pt")
                        if diag:
                            nc.tensor.matmul(sp[:, r0:QB], kt_sb[bsl, pair, j2 * KB:(j2 + 1) * KB],
                                             qrhs[:, r0:], start=True, stop=True)
                            nc.tensor.matmul(sp[:, QB:QB + w1],
                                             kt_sb[bsl, pair, (j2 + 1) * KB:(j2 + 2) * KB],
                                             qrhs[:, r1:], start=True, stop=True)
                            mm((QB - r0 + w1) * PE_NS)
                            nc.scalar.activation(out=pt[:, r0:QB + w1], in_=sp[:, r0:QB + w1],
                                                 func=EXP, scale=SCALE / (WSCALE * WSCALE))
                            clk["act"] = max(clk["act"], clk["pe"] + o["sem_lat"]) \
                                + (QB - r0 + w1) * ACT_NS + EXP_OVH
                        else:
                            for jj in range(2):
                                j = j2 + jj
                                nc.tensor.matmul(sp[:, jj * QB:(jj + 1) * QB],
                                                 kt_sb[bsl, pair, j * KB:(j + 1) * KB],
                                                 qrhs, start=True, stop=True)
                            mm(2 * QB * PE_NS)
                            nc.scalar.activation(out=pt[:], in_=sp[:], func=EXP,
                                                 scale=SCALE / (WSCALE * WSCALE))
                            clk["act"] = max(clk["act"], clk["pe"] + o["sem_lat"]) \
                                + 2 * QB * ACT_NS + EXP_OVH
                        return pt

                    def oo(pt):
                        if diag:
                            nc.vector.tensor_mul(pt[:, r0:r0 + KB], pt[:, r0:r0 + KB], tri_sb[:])
                            nc.vector.tensor_mul(pt[:, QB:QB + KB], pt[:, QB:QB + KB], tri_sb[:])
                            nc.tensor.matmul(o_p[:, r0:QB], v_sb[:, j2, slot(hb)],
                                             pt[:, r0:QB], start=(j2 == 0), stop=False)
                            nc.tensor.matmul(o_p[:, r1:QB], v_sb[:, j2 + 1, slot(hb)],
                                             pt[:, QB:QB + w1], start=False,
                                             stop=(j2 + 1 == nk - 1))
                            mm((QB - r0 + w1) * PE_NS)
                        else:
                            for jj in range(2):
                                j = j2 + jj
                                nc.tensor.matmul(o_p[:], v_sb[:, j, slot(hb)],
                                                 pt[:, jj * QB:(jj + 1) * QB],
                                                 start=(j == 0), stop=False)
                            mm(2 * QB * PE_NS)
                    return ss, oo

                groups = [grp(j2, False) for j2 in range(0, nfull, 2)] \
                    + [grp(j2, True) for j2 in range(nfull, nk, 2)]
                # software-pipelined: SS/exp of group g+1 runs before OO of g
                # so the next S matmuls (plus fillers) cover the exp latency
                prev = None
                for gi, (ss, oo) in enumerate(groups):
                    if gi == len(groups) - 2 and prediag is not None:
                        prediag()
                    pt = ss()
                    pace(clk["act"] + o["ret_lat"])
                    if prev is not None:
                        prev[1](prev[0])
                    prev = (pt, oo)
                prev[1](prev[0])
                normalize(o_p, onorm, h, splits)
                pace(clk["pe"] + o["end_fill"])

            # ---- main schedule ----------------------------------------------
            emit_x_dma(0)
            nc.vector.tensor_copy(out=v_ones, in_=ones_sb.rearrange("p (n o) -> p n o", o=1))

            pending_y = []
            for qb in range(NQB):
                if qb == 0:
                    fillers.extend((f"{t}@0", c, f, a) for t, c, f, a in q_units(0))
                    fillers.extend((f"{t}@0", c, f, a) for t, c, f, a in kv_units(0))
                if 0 < qb < NQB - 1:
                    emit_x_dma(qb + 1)
                # this phase's Q (and for streams' diagonals, K/V) must be
                # emitted before the attention that reads it
                drain(lambda t: t == f"qp0@{qb}")
                if qb < NQB - 1:
                    fillers.extend((f"{t}@{qb + 1}", c, f, a) for t, c, f, a in q_units(qb + 1))
                    fillers.extend((f"{t}@{qb + 1}", c, f, a) for t, c, f, a in kv_units(qb + 1))
                onorms = [onpool.tile([128, QB], BF16, tag="on", name=f"on{qb}_{i}") for i in range(2)]
                splits = o["norm_splits_last"] if qb == NQB - 1 else o["norm_splits"]
                for si, (pair, h) in enumerate(((0, 0), (0, 1), (1, 0), (1, 1))):
                    if pair == 1 and h == 0:
                        drain(lambda t: t == f"qp1@{qb}")
                    prediag = (lambda p=pair: drain(
                        lambda t: t in (f"kp{p}@{qb}", f"v@{qb}"))) if h == 0 else None
                    stream(qb, pair, h, onorms[pair], splits, prediag=prediag)
                    if si == 0 and pending_y and (
                            qb == NQB - 1 or o["y_defer"] == 1):
                        fillers.extend(pending_y)
                        pending_y = []
                pending_y = pending_y + y_units(qb, onorms)
            while fillers:
                emit_one()
            for _, _, fn, _a in pending_y:
                fn()

    nc.compile()
    return nc


_NC = {}


def _get_program(opts=None):
    key = tuple(sorted((opts or {}).items()))
    if key not in _NC:
        _NC[key] = build_program(opts)
    return _NC[key]


def _make_in_maps(x, Wq, Wk, Wv, Wp):
    bf = ml_dtypes.bfloat16
    f8 = ml_dtypes.float8_e4m3
    x32 = np.asarray(x, np.float32)
    xh = x32.astype(f8)
    xl = (x32 - xh.astype(np.float32)).astype(f8)

    def wsplit(W):
        wt = np.asarray(W, np.float32).T * WSCALE
        hi = wt.astype(f8)
        lo = (wt - hi.astype(np.float32)).astype(f8)
        return hi, lo

    wqh, wql = wsplit(Wq)
    wkh, wkl = wsplit(Wk)
    wvh, wvl = wsplit(Wv)
    wpt = np.ascontiguousarray((np.asarray(Wp, np.float32).T / WSCALE).astype(bf))
    tri = (np.arange(KB)[:, None] <= np.arange(KB)[None, :]).astype(bf)
    ones = np.ones((128, (T // KB) * GH), bf)
    in_maps = []
    for c in range(N_CORES):
        b, hg = c // 4, c % 4
        gsl = slice(hg * GE, (hg + 1) * GE)
        in_maps.append({
            "xh": np.ascontiguousarray(xh[b].T),
            "xl": np.ascontiguousarray(xl[b].T),
            "wqh": np.ascontiguousarray(wqh[:, gsl]),
            "wql": np.ascontiguousarray(wql[:, gsl]),
            "wkh": np.ascontiguousarray(wkh[:, gsl]),
            "wkl": np.ascontiguousarray(wkl[:, gsl]),
            "wvh": np.ascontiguousarray(wvh[:, gsl]),
            "wvl": np.ascontiguousarray(wvl[:, gsl]),
            "wpt": np.ascontiguousarray(wpt[gsl, :]),
            "tri": tri,
            "ones": ones,
        })
    return in_maps


def run_cores(x, Wq, Wk, Wv, Wp, bp, **spmd_kwargs):
    """Run the 8-core program; returns (y_full, BassKernelResults)."""
    nc = _get_program()
    in_maps = _make_in_maps(x, Wq, Wk, Wv, Wp)
    res = run_bass_kernel_spmd(nc, in_maps, list(range(N_CORES)), **spmd_kwargs)
    parts = [res.results[c]["y"] for c in range(N_CORES)]
    y = np.empty((B, T, E), np.float32)
    for b in range(B):
        acc = parts[4 * b].astype(np.float32)
        for hg in range(1, 4):
            acc = acc + parts[4 * b + hg].astype(np.float32)
        y[b] = acc
    y += np.asarray(bp, np.float32)[None, None, :]
    return y, res


def kernel(x, Wq, Wk, Wv, Wp, bp):
    y, _ = run_cores(x, Wq, Wk, Wv, Wp, bp)
    return y


def bench(x, Wq, Wk, Wv, Wp, bp, iters=12):
    """Time repeated on-device executions of the compiled program."""
    import time

    import jax
    import numpy as np_
    from jax.experimental.shard_map import shard_map
    from jax.sharding import Mesh, NamedSharding, PartitionSpec

    from concourse import bass2jax, mybir as mb

    nc = _get_program()
    in_maps = _make_in_maps(x, Wq, Wk, Wv, Wp)
    n_cores = N_CORES
    bass2jax.install_neuronx_cc_hook()

    partition_name = nc.partition_id_tensor.name if nc.partition_id_tensor else None
    in_names, out_names, out_avals, zero_outs = [], [], [], []
    for alloc in nc.m.functions[0].allocations:
        if not isinstance(alloc, mb.MemoryLocationSet):
            continue
        name = alloc.memorylocations[0].name
        if alloc.kind == "ExternalInput":
            if name != partition_name:
                in_names.append(name)
        elif alloc.kind == "ExternalOutput":
            out_names.append(name)
            shape = tuple(alloc.tensor_shape)
            dtype = mb.dt.np(alloc.dtype)
            out_avals.append(jax.core.ShapedArray(shape, dtype))
            zero_outs.append(np_.zeros(shape, dtype))
    n_params = len(in_names)
    all_in_names = in_names + out_names
    if partition_name is not None:
        all_in_names = all_in_names + [partition_name]

    def _body(*args):
        operands = list(args)
        if partition_name is not None:
            operands.append(bass2jax.partition_id_tensor())
        outs = bass2jax._bass_exec_p.bind(
            *operands,
            out_avals=tuple(out_avals),
            in_names=tuple(all_in_names),
            out_names=tuple(out_names),
            lowering_input_output_aliases=(),
            sim_require_finite=True,
            sim_require_nnan=True,
            nc=nc,
        )
        return tuple(outs)

    devices = jax.devices()[:n_cores]
    mesh = Mesh(np_.asarray(devices), ("core",))
    donate = tuple(range(n_params, n_params + len(out_names)))
    sharded = jax.jit(
        shard_map(_body, mesh=mesh,
                  in_specs=(PartitionSpec("core"),) * (n_params + len(out_names)),
                  out_specs=(PartitionSpec("core"),) * len(out_names),
                  check_rep=False),
        donate_argnums=donate, keep_unused=True,
    )
    sh = NamedSharding(mesh, PartitionSpec("core"))
    concat_in = [
        jax.device_put(
            np_.concatenate([np_.asarray(in_maps[c][nm]) for c in range(n_cores)], axis=0), sh)
        for nm in in_names
    ]
    zeros_np = [np_.zeros((n_cores * z.shape[0], *z.shape[1:]), z.dtype) for z in zero_outs]

    times = []
    out_arrs = None
    for it in range(iters):
        dz = [jax.device_put(z, sh) for z in zeros_np]
        jax.block_until_ready(dz)
        t0 = time.perf_counter()
        out_arrs = sharded(*concat_in, *dz)
        jax.block_until_ready(out_arrs)
        times.append(time.perf_counter() - t0)

    parts = [
        np_.asarray(out_arrs[i]).reshape(n_cores, *out_avals[i].shape)
        for i, nm in enumerate(out_names)
    ]
    yi = out_names.index("y")
    y = np_.empty((B, T, E), np_.float32)
    for b in range(B):
        acc = parts[yi][4 * b].astype(np_.float32)
        for hg in range(1, 4):
            acc = acc + parts[yi][4 * b + hg].astype(np_.float32)
        y[b] = acc
    y += np_.asarray(bp, np_.float32)[None, None, :]
    return y, times



# revision 37
# speedup vs baseline: 1.1938x; 1.1938x over previous
"""Multi-head causal attention (B=2, T=2048, E=1024, H=16, D=64) on 8 trn2 cores.

Sharding: core c -> batch b = c // 4, head-group hg = c % 4 (4 heads each).
Per-core: QKV projections for its 4 heads, causal flash attention in
transposed-score layout (S^T[k,q]; softmax denominator folded into a
ones-augmented V matmul), row-parallel output projection producing a partial
[T, E] output. Host sums the 4 partials per batch and adds the bias.

v3: all attention matmuls in fp8 DoubleRow mode (0.5 cyc/col):
 - S = K^T.Q: lhsT ktiles = (zeros-chunk, K-chunk) within one kt tile
   (step-sliced AP), rhs = Q broadcast stride-0 over the ktile dim.
   Q,K quantized to fp8e4m3 (single term x_hi*w_hi projections).
 - P.V: lhsT ktiles = (V_hi, V_lo) fp8 split (exact to ~0.03%), rhs = P
   (exp output written directly as fp8) broadcast stride-0.
 - Q/K projections 1-term (fp8 rounding dominates anyway); V stays 3-term
   hi/lo since V error enters the output linearly (small-neff rows).
Causal masking stays post-exp via tri-mask multiplies, moved to gpsimd.
Output projection stays bf16. qk drain copies moved to DVE (ACT is the
bottleneck: exp cols ~58us + per-instr overhead).
"""
import collections
import sys
from contextlib import ExitStack

sys.path.insert(0, "/opt/trn_rl_repo")

import ml_dtypes
import numpy as np

import concourse.bass as bass
import concourse.tile as tile
from concourse import bacc, mybir
from concourse.bass_utils import run_bass_kernel_spmd

F32 = mybir.dt.float32
BF16 = mybir.dt.bfloat16
FP8 = mybir.dt.float8e4
DR = mybir.MatmulPerfMode.DoubleRow
EXP = mybir.ActivationFunctionType.Exp
SUB = mybir.AluOpType.subtract
MUL = mybir.AluOpType.mult

WSCALE = 32.0           # host prescale on Wq/Wk/Wv for fp8 range; q,k,v come
                        # out x32, folded into the exp scale and into Wp

B, T, E, H = 2, 2048, 1024, 16
D = E // H              # 64
N_CORES = 8
GH = 4                  # heads per core
GE = GH * D             # 256 per-core projection width
SCALE = float(D) ** -0.5

TCH = 512               # projection t-chunk
NTCH = T // TCH         # 4
KC = 8                  # contraction chunks of 128 over E
KC2 = KC // 2
QB = 512                # attention q-block
NQB = T // QB           # 4
KB = 128                # attention k-block
NKB = T // KB           # 16
VSLOT = 80              # v8 per-head slot width (64 data + ones col + pad, %16)

PE_NS = 1e9 / 2.4e9     # per moving-free column (bf16)
ACT_NS = 1e9 / 1.2e9    # per free column
EXP_OVH = 217.0         # ACT per-instruction overhead (access + decode)

DEFAULT_OPTS = dict(
    s_bufs=2,
    o_bufs=2,
    pv_bufs=2,
    p_bufs=6,
    x_bufs=4,
    l_bufs=6,
    on_bufs=8,
    yst_bufs=4,
    norm_splits=1,       # normalize split count (qb < last)
    norm_splits_last=2,  # normalize split count for the last q-block
    sem_lat=400.0,       # pacing fudge: SS-end -> exp-start latency
    ret_lat=1000.0,      # pacing fudge: exp-end -> O-start latency
    lead=0.0,            # pacing margin (ns)
    end_fill=1200.0,     # filler ns pulled at each stream end (norm window)
    copy_cd=1100.0,       # ns between a proj drain copy and the next slot alloc
    y_defer=1,           # 1: Y(qb) paced into phase qb+1; 3: all saved for last phase
    qk_copy_eng="vector",
    qk_terms=3,          # x*W terms for Q/K proj (1=hh only; 3=hh,lh,hl)
    tri_eng="vector",    # engine for the post-exp causal masking multiplies
    y_tail_split=True,   # alternate last-phase y drains between DVE and ACT
    early_qp0=True,      # drain next phase's qp0 at pair-1 start (DVE queue
                         # order: its copy lands before the phase-end bursts)
    sel_drain=True,      # drain() defers y units instead of emitting them
    cd_hist=1,           # which drain-copy (1=last) gates the next slot alloc
    early_act_copies=2,  # tch < this: qk/v drain copies go to ACT (idle early)
    warmup=80,           # junk DR matmuls at t=0 to climb the PE p-state ramp
)


def build_program(opts=None):
    o = dict(DEFAULT_OPTS)
    if opts:
        o.update(opts)
    nc = bacc.Bacc("TRN2", target_bir_lowering=False, debug=False, num_devices=N_CORES)

    xh_d = nc.dram_tensor("xh", [E, T], FP8, kind="ExternalInput").ap()
    xl_d = nc.dram_tensor("xl", [E, T], FP8, kind="ExternalInput").ap()
    wqh_d = nc.dram_tensor("wqh", [E, GE], FP8, kind="ExternalInput").ap()
    wql_d = nc.dram_tensor("wql", [E, GE], FP8, kind="ExternalInput").ap()
    wkh_d = nc.dram_tensor("wkh", [E, GE], FP8, kind="ExternalInput").ap()
    wkl_d = nc.dram_tensor("wkl", [E, GE], FP8, kind="ExternalInput").ap()
    wvh_d = nc.dram_tensor("wvh", [E, GE], FP8, kind="ExternalInput").ap()
    wvl_d = nc.dram_tensor("wvl", [E, GE], FP8, kind="ExternalInput").ap()
    wpt_d = nc.dram_tensor("wpt", [GE, E], BF16, kind="ExternalInput").ap()
    tri_d = nc.dram_tensor("tri", [KB, KB], FP8, kind="ExternalInput").ap()
    y_d = nc.dram_tensor("y", [T, E], BF16, kind="ExternalOutput").ap()

    with tile.TileContext(nc) as tc:
        with tc.tile_pool(name="weights", bufs=1) as wpool, \
             tc.tile_pool(name="qk", bufs=1) as qkpool, \
             tc.tile_pool(name="vsb", bufs=1) as vpool, \
             tc.tile_pool(name="xin", bufs=o["x_bufs"]) as xpool, \
             tc.tile_pool(name="ptile", bufs=o["p_bufs"]) as ppool, \
             tc.tile_pool(name="lbc", bufs=o["l_bufs"]) as lpool, \
             tc.tile_pool(name="onorm", bufs=o["on_bufs"]) as onpool, \
             tc.tile_pool(name="ystage", bufs=o["yst_bufs"]) as ypool, \
             tc.tile_pool(name="s_ps", bufs=o["s_bufs"], space="PSUM") as s_ps, \
             tc.tile_pool(name="pv_ps", bufs=o["pv_bufs"], space="PSUM") as pv_ps, \
             tc.tile_pool(name="o_ps", bufs=o["o_bufs"], space="PSUM") as o_ps:
            qk_ps = v_ps = pv_ps

            nqk = 2 if o["qk_terms"] > 1 else 1
            wq_sb = [wpool.tile([128, KC2, 2, GE], FP8, name=f"wq{i}") for i in range(nqk)]
            wk_sb = [wpool.tile([128, KC2, 2, GE], FP8, name=f"wk{i}") for i in range(nqk)]
            wv_sb = [wpool.tile([128, KC2, 2, GE], FP8, name=f"wv{i}") for i in range(2)]
            wp_sb = wpool.tile([128, 2, E], BF16)
            tri_sb = wpool.tile([KB, KB], FP8)

            # Q^T per pair: [p=(h%2,d), pair, t] fp8
            qt_sb = qkpool.tile([128, 2, T], FP8)
            # K^T chunked: chunks 0 and 33 = zeros (DR ktile partner / warmup
            # operands), chunks 1+pair*16+j = K^T[:, j*128:(j+1)*128]
            kt_sb = qkpool.tile([128, 2 * NKB + 2, KB], FP8)
            # V: [p=key%128, tblock, hi/lo, head*80 + (d | ones at 64 | pad)]
            v_sb = vpool.tile([128, NKB, 2, GH * VSLOT], FP8)

            xts = [None] * NTCH  # per-tch ([hi, lo]) [128, KC2, 2, TCH] tiles

            def dr(ap3):
                # dram [rows, n] -> DoubleRow sbuf layout [p, c2, i, n]
                return ap3.rearrange("(c i p) n -> p c i n", i=2, p=128)

            def w_dma(w_sb_t, w_d):
                nc.sync.dma_start(out=w_sb_t[:], in_=dr(w_d))

            def emit_x_dma(tch):
                xts[tch] = [xpool.tile([128, KC2, 2, TCH], FP8, tag="xt",
                                       name=f"x{tch}_{hl}") for hl in range(2)]
                xsrc = [x_d[:, tch * TCH:(tch + 1) * TCH] for x_d in (xh_d, xl_d)]
                if tch == 0:
                    # prologue: HWDGE serializes DMA issue at ~650ns each, so
                    # one DMA per tensor, ordered by first use: the pair-0 Q/K
                    # projection chain gates the first exp.
                    nc.sync.dma_start(out=xts[0][0][:], in_=dr(xsrc[0]))
                    w_dma(wq_sb[0], wqh_d)
                    if nqk > 1:
                        nc.sync.dma_start(out=xts[0][1][:], in_=dr(xsrc[1]))
                        w_dma(wq_sb[1], wql_d)
                    w_dma(wk_sb[0], wkh_d)
                    if nqk > 1:
                        w_dma(wk_sb[1], wkl_d)
                    w_dma(wv_sb[0], wvh_d)
                    w_dma(wv_sb[1], wvl_d)
                    if nqk == 1:
                        nc.sync.dma_start(out=xts[0][1][:], in_=dr(xsrc[1]))
                    nc.sync.dma_start(out=tri_sb[:], in_=tri_d)
                    emit_x_dma(1)
                    nc.sync.dma_start(
                        out=wp_sb[:], in_=wpt_d.rearrange("(c p) n -> p c n", p=128))
                else:
                    for hl in range(2):
                        nc.sync.dma_start(out=xts[tch][hl][:], in_=dr(xsrc[hl]))

            # ---- pacing state ------------------------------------------------
            clk = {"pe": 0.0, "act": 0.0, "allow_y": False}
            copy_hist = collections.deque([-1e9] * 8, maxlen=8)
            fillers = collections.deque()    # proj units (tag, pe_ns, fn, allocs)
            fillers_y = collections.deque()  # y units: only emitted mid-stream

            def mm(pe_ns):
                clk["pe"] += pe_ns

            def emit_from(q):
                tag, pe_ns, fn, _alloc = q.popleft()
                marks = fn() or ()
                clk["pe"] += pe_ns
                if "copy" in marks:
                    copy_hist.append(clk["pe"])

            def emit_one():
                emit_from(fillers)

            def cd_blocked():
                # hold back a unit that re-allocates a shared proj psum slot
                # until the drain copy cd_hist groups back had time to land
                return clk["pe"] < copy_hist[-o["cd_hist"]] + o["copy_cd"]

            def pace(target):
                # proj fillers first; y units only mid-stream (a y matmul
                # stalls the in-order PE queue on its DVE drain copy, so they
                # must never sit ahead of a phase's first S matmuls)
                while clk["pe"] < target - o["lead"]:
                    if fillers and not (fillers[0][3] and cd_blocked()):
                        emit_from(fillers)
                    elif fillers_y and clk["allow_y"]:
                        emit_from(fillers_y)
                    else:
                        break

            def drain(tag_pred):
                while any(tag_pred(t) for t, _, _, _ in fillers):
                    emit_one()

            def qk_drain_copy(dst, src, tch=99):
                if o["qk_copy_eng"] == "scalar" or tch < o["early_act_copies"]:
                    nc.scalar.copy(out=dst, in_=src)
                else:
                    nc.vector.tensor_copy(out=dst, in_=src)

            # ---- projection units -------------------------------------------
            TERMS = ((0, 0), (1, 0), (0, 1))  # (w hi/lo, x hi/lo): hh, lh, hl

            QK_TERMS = TERMS[:o["qk_terms"]]

            def qk_mms(ph, w_sb, tch, pair, c2):
                for ti, (wl, xl) in enumerate(QK_TERMS):
                    nc.tensor.matmul(
                        ph[:],
                        w_sb[wl][:, c2, :, pair * 128:(pair + 1) * 128],
                        xts[tch][xl][:, c2, :, :],
                        start=(c2 == 0 and ti == 0),
                        stop=(c2 == KC2 - 1 and ti == len(QK_TERMS) - 1),
                        perf_mode=DR)

            QKC = (TCH // 2) * PE_NS * o["qk_terms"]  # pe-ns per qk unit

            def q_units(tch):
                qp_h = {}

                def q_u(pair, c2):
                    def fn():
                        if c2 == 0:
                            qp_h[pair] = qk_ps.tile([128, TCH], F32, tag="pv",
                                                    name=f"q_{tch}_{pair}")
                        qk_mms(qp_h[pair], wq_sb, tch, pair, c2)
                        if c2 == KC2 - 1:
                            qk_drain_copy(
                                qt_sb[:, pair, tch * TCH:(tch + 1) * TCH],
                                qp_h[pair][:], tch)
                            return ("copy",)
                    return fn
                return [(f"qp{pair}", QKC, q_u(pair, c2), c2 == 0)
                        for pair in range(2) for c2 in range(KC2)]

            def kv_units(tch):
                kp_h = {}

                def k_u(pair, c2):
                    def fn():
                        if c2 == 0:
                            kp_h[pair] = qk_ps.tile([128, TCH], F32, tag="pv",
                                                    name=f"k_{tch}_{pair}")
                        qk_mms(kp_h[pair], wk_sb, tch, pair, c2)
                        if c2 == KC2 - 1:
                            ch = 1 + pair * NKB + tch * (TCH // KB)
                            qk_drain_copy(
                                kt_sb[:, ch:ch + TCH // KB, :],
                                kp_h[pair].rearrange("p (c n) -> p c n", n=KB), tch)
                            return ("copy",)
                    return fn

                units = [(f"kp{pair}", QKC, k_u(pair, c2), c2 == 0)
                         for pair in range(2) for c2 in range(KC2)]
                vp_h = {}

                def v_u(tsub, half):
                    def fn():
                        if half == 0:
                            vp_h[tsub] = v_ps.tile([128, GE], F32, tag="pv",
                                                   name=f"vp{tch}_{tsub}")
                        for c2 in range(2 * half, 2 * half + 2):
                            for ti, (wl, xl) in enumerate(TERMS):
                                nc.tensor.matmul(
                                    vp_h[tsub][:],
                                    xts[tch][xl][:, c2, :, tsub * KB:(tsub + 1) * KB],
                                    wv_sb[wl][:, c2, :, :],
                                    start=(c2 == 0 and ti == 0),
                                    stop=(c2 == KC2 - 1 and ti == len(TERMS) - 1),
                                    perf_mode=DR)
                        if half == 1:
                            tb = tch * (TCH // KB) + tsub
                            vsrc = vp_h[tsub].rearrange("p (h c) -> p h c", c=D)
                            vhi = v_sb.rearrange(
                                "p b i (h w) -> p b i h w", w=VSLOT)[:, tb, 0, :, 0:D]
                            vlo = v_sb.rearrange(
                                "p b i (h w) -> p b i h w", w=VSLOT)[:, tb, 1, :, 0:D]
                            if tch < o["early_act_copies"]:
                                nc.scalar.copy(out=vhi, in_=vsrc)
                            else:
                                nc.vector.tensor_copy(out=vhi, in_=vsrc)
                            nc.vector.tensor_tensor(out=vlo, in0=vsrc, in1=vhi, op=SUB)
                            return ("copy",)
                    return fn

                # shared proj psum slots: groups must stay contiguous
                k0 = [u for u in units if u[0] == "kp0"]
                k1 = [u for u in units if u[0] == "kp1"]
                vs = [("v", 3 * GE * PE_NS, v_u(t, half), half == 0)
                      for t in range(TCH // KB) for half in range(2)]
                return k0, vs, k1

            # ---- output-projection units ------------------------------------
            def y_units(qb, onorms):
                q0 = qb * QB
                units = []
                yt_h = {}
                late = qb >= NQB - 2  # runs in phase 3 / tail: pv pool is idle

                def y_unit(qt, nh):
                    def fn():
                        if nh == 0:
                            yt_h[qt] = ypool.tile([128, E], BF16, tag="yt", name=f"yt{qt}")
                        if late:
                            yp = pv_ps.tile([128, 512], F32, tag="pv", name="yp")
                        else:
                            yp = s_ps.tile([128, 512], F32, tag="s", name="yp")
                        for pair in range(2):
                            nc.tensor.matmul(yp[:],
                                             onorms[pair][:, qt * 128:(qt + 1) * 128],
                                             wp_sb[:, pair, nh * 512:(nh + 1) * 512],
                                             start=(pair == 0), stop=(pair == 1))
                        ysl = yt_h[qt][:, nh * 512:(nh + 1) * 512]
                        if qb == NQB - 1 and o["y_tail_split"] and (qt + nh) % 2:
                            nc.scalar.copy(out=ysl, in_=yp[:])
                        else:
                            nc.vector.tensor_copy(out=ysl, in_=yp[:])
                        nc.sync.dma_start(
                            out=y_d[q0 + qt * 128:q0 + (qt + 1) * 128, nh * 512:(nh + 1) * 512],
                            in_=ysl)
                    return fn

                for qt in range(QB // 128):
                    for nh in range(2):
                        units.append(("y", 2 * 512 * PE_NS, y_unit(qt, nh), False))
                return units

            # ---- attention stream -------------------------------------------
            def vslot(hb, hl):
                # [128, 2, 65] hi/lo ktile view of head hb at key-block j
                def at(j):
                    base = v_sb.rearrange("p b i (h w) -> p b i h w", w=VSLOT)
                    return base[:, j, :, hb, 0:D + 1]
                return at

            def kdr(pair, j, h):
                # lhsT [64, 2, 128]: ktile 0 = zeros chunk, ktile 1 = K chunk
                c = 1 + pair * NKB + j
                return kt_sb[h * D:(h + 1) * D, 0:c + 1:c, :]

            def bcast2(ap):
                return ap.unsqueeze(1).broadcast_to([ap.shape[0], 2, ap.shape[1]])

            def bcast2p(ap):
                return ap.unsqueeze(1).broadcast_to([128, 2, ap.shape[1]])

            def normalize(o_p, onorm, h, splits=1):
                w = QB // splits
                for s in range(splits):
                    qs = slice(s * w, (s + 1) * w)
                    strip = lpool.tile([1, w], F32, tag="strip")
                    nc.vector.reciprocal(out=strip[:], in_=o_p[D:D + 1, qs])
                    lb = lpool.tile([D, w], F32, tag="lb")
                    nc.gpsimd.partition_broadcast(lb[:], strip[:])
                    nc.vector.tensor_mul(onorm[h * D:(h + 1) * D, qs], o_p[0:D, qs], lb[:])

            def stream(qb, pair, h, onorm, splits, prek=None, prev_v=None):
                q0 = qb * QB
                nk = (q0 + QB) // KB
                nfull = nk - 4
                bsl = slice(h * D, h * D + D)
                hb = pair * 2 + h
                vat = vslot(hb, 0)
                o_p = o_ps.tile([D + 1, QB], F32, tag="o")
                qrhs = qt_sb[bsl, pair, q0:q0 + QB]

                def grp(j2, diag):
                    r0 = (j2 - nfull) * KB if diag else 0
                    r1 = r0 + KB
                    w1 = QB - r1

                    def ss():
                        sp = s_ps.tile([128, 2 * QB], F32, tag="s", name="sp")
                        pt = ppool.tile([128, 2 * QB], FP8, tag="p", name="pt")
                        if diag:
                            nc.tensor.matmul(sp[:, r0:QB], kdr(pair, j2, h),
                                             bcast2(qrhs[:, r0:]),
                                             start=True, stop=True, perf_mode=DR)
                            nc.tensor.matmul(sp[:, QB:QB + w1], kdr(pair, j2 + 1, h),
                                             bcast2(qrhs[:, r1:]),
                                             start=True, stop=True, perf_mode=DR)
                            mm((QB - r0 + w1) * PE_NS / 2)
                            nc.scalar.activation(out=pt[:, r0:QB + w1], in_=sp[:, r0:QB + w1],
                                                 func=EXP, scale=SCALE / (WSCALE * WSCALE))
                            clk["act"] = max(clk["act"], clk["pe"] + o["sem_lat"]) \
                                + (QB - r0 + w1) * ACT_NS + EXP_OVH
                        else:
                            for jj in range(2):
                                j = j2 + jj
                                nc.tensor.matmul(sp[:, jj * QB:(jj + 1) * QB],
                                                 kdr(pair, j, h), bcast2(qrhs),
                                                 start=True, stop=True, perf_mode=DR)
                            mm(2 * QB * PE_NS / 2)
                            nc.scalar.activation(out=pt[:], in_=sp[:], func=EXP,
                                                 scale=SCALE / (WSCALE * WSCALE))
                            clk["act"] = max(clk["act"], clk["pe"] + o["sem_lat"]) \
                                + 2 * QB * ACT_NS + EXP_OVH
                        return pt

                    def oo(pt):
                        if diag:
                            tri_e = nc.gpsimd if o["tri_eng"] == "pool" else nc.vector
                            tri_e.tensor_tensor(out=pt[:, r0:r0 + KB],
                                                in0=pt[:, r0:r0 + KB], in1=tri_sb[:], op=MUL)
                            tri_e.tensor_tensor(out=pt[:, QB:QB + KB],
                                                in0=pt[:, QB:QB + KB], in1=tri_sb[:], op=MUL)
                            nc.tensor.matmul(o_p[:, r0:QB], vat(j2),
                                             bcast2p(pt[:, r0:QB]),
                                             start=(j2 == 0), stop=False, perf_mode=DR)
                            nc.tensor.matmul(o_p[:, r1:QB], vat(j2 + 1),
                                             bcast2p(pt[:, QB:QB + w1]),
                                             start=False, stop=(j2 + 1 == nk - 1),
                                             perf_mode=DR)
                            mm((QB - r0 + w1) * PE_NS / 2)
                        else:
                            for jj in range(2):
                                j = j2 + jj
                                nc.tensor.matmul(o_p[:], vat(j),
                                                 bcast2p(pt[:, jj * QB:(jj + 1) * QB]),
                                                 start=(j == 0), stop=False, perf_mode=DR)
                            mm(2 * QB * PE_NS / 2)
                    return ss, oo

                groups = [grp(j2, False) for j2 in range(0, nfull, 2)] \
                    + [grp(j2, True) for j2 in range(nfull, nk, 2)]
                # software-pipelined: SS/exp of group g+1 runs before OO of g
                # so the next S matmuls (plus fillers) cover the exp latency
                prev = None
                ndiag = len(groups) - 2  # first group whose SS reads this tch's K
                for gi, (ss, oo) in enumerate(groups):
                    if gi == ndiag and prek is not None:
                        prek()
                    pt = ss()
                    pace(clk["act"] + o["ret_lat"])
                    if prev is not None:
                        if prev[2] == ndiag and prev_v is not None:
                            prev_v()
                        prev[1](prev[0])
                    prev = (pt, oo, gi)
                if prev[2] == ndiag and prev_v is not None:
                    prev_v()
                prev[1](prev[0])
                normalize(o_p, onorm, h, splits)
                pace(clk["pe"] + o["end_fill"])

            # ---- main schedule ----------------------------------------------
            nc.vector.memset(kt_sb[:, 0, :], 0.0)
            nc.vector.memset(kt_sb[:, 2 * NKB + 1, :], 0.0)
            if o["warmup"]:
                # climb the PE p-state ramp on junk zero matmuls while the
                # prologue DMAs land; results are discarded
                wu_ps = s_ps.tile([128, 2 * QB], F32, tag="s", name="wu")
                wu_lhs = kt_sb[0:64, 0::(2 * NKB + 1), :]
                wu_rhs = kt_sb[0:64, 0, :].unsqueeze(1).broadcast_to([64, 2, KB])
                for _ in range(o["warmup"]):
                    nc.tensor.matmul(wu_ps[:, 0:KB], wu_lhs, wu_rhs,
                                     start=True, stop=True, perf_mode=DR)
            emit_x_dma(0)
            vv = v_sb.rearrange("p b i (h w) -> p b i h w", w=VSLOT)
            nc.vector.memset(vv[:, :, 0, :, D:D + 1], 1.0)
            nc.vector.memset(vv[:, :, 1, :, D:D + 1], 0.0)

            def queue_proj(tch):
                qs = q_units(tch)
                q0 = [u for u in qs if u[0] == "qp0"]
                q1 = [u for u in qs if u[0] == "qp1"]
                k0, vs, k1 = kv_units(tch)
                for t, c, f, a in q0 + k0 + vs + q1 + k1:
                    fillers.append((f"{t}@{tch}", c, f, a))

            pending_y = []
            for qb in range(NQB):
                if qb == 0:
                    queue_proj(0)
                if 0 < qb < NQB - 1:
                    emit_x_dma(qb + 1)
                # this phase's Q (and for streams' diagonals, K/V) must be
                # emitted before the attention that reads it
                drain(lambda t: t == f"qp0@{qb}")
                if qb < NQB - 1:
                    queue_proj(qb + 1)
                onorms = [onpool.tile([128, QB], BF16, tag="on", name=f"on{qb}_{i}") for i in range(2)]
                splits = o["norm_splits_last"] if qb == NQB - 1 else o["norm_splits"]
                for si, (pair, h) in enumerate(((0, 0), (0, 1), (1, 0), (1, 1))):
                    clk["allow_y"] = si < 3 or qb == NQB - 1
                    if pair == 1 and h == 0:
                        drain(lambda t: t == f"qp1@{qb}")
                        if o["early_qp0"] and qb < NQB - 1:
                            drain(lambda t: t == f"qp0@{qb + 1}")
                    prek = (lambda p=pair: drain(
                        lambda t: t == f"kp{p}@{qb}")) if h == 0 else None
                    prev_v = (lambda: drain(
                        lambda t: t == f"v@{qb}")) if (pair == 0 and h == 0) else None
                    stream(qb, pair, h, onorms[pair], splits, prek=prek, prev_v=prev_v)
                    if si == 0 and pending_y and (
                            qb == NQB - 1 or o["y_defer"] == 1):
                        fillers_y.extend(pending_y)
                        pending_y = []
                pending_y = pending_y + y_units(qb, onorms)
            clk["allow_y"] = True
            while fillers or fillers_y:
                emit_from(fillers if fillers else fillers_y)
            for _, _, fn, _a in pending_y:
                fn()

    nc.compile()
    return nc


_NC = {}


def _get_program(opts=None):
    key = tuple(sorted((opts or {}).items()))
    if key not in _NC:
        _NC[key] = build_program(opts)
    return _NC[key]


def _make_in_maps(x, Wq, Wk, Wv, Wp):
    bf = ml_dtypes.bfloat16
    f8 = ml_dtypes.float8_e4m3
    x32 = np.asarray(x, np.float32)
    xh = x32.astype(f8)
    xl = (x32 - xh.astype(np.float32)).astype(f8)

    def wsplit(W):
        wt = np.asarray(W, np.float32).T * WSCALE
        hi = wt.astype(f8)
        lo = (wt - hi.astype(np.float32)).astype(f8)
        return hi, lo

    wqh, wql = wsplit(Wq)
    wkh, wkl = wsplit(Wk)
    wvh, wvl = wsplit(Wv)
    wpt = np.ascontiguousarray((np.asarray(Wp, np.float32).T / WSCALE).astype(bf))
    tri = (np.arange(KB)[:, None] <= np.arange(KB)[None, :]).astype(f8)
    in_maps = []
    for c in range(N_CORES):
        b, hg = c // 4, c % 4
        gsl = slice(hg * GE, (hg + 1) * GE)
        in_maps.append({
            "xh": np.ascontiguousarray(xh[b].T),
            "xl": np.ascontiguousarray(xl[b].T),
            "wqh": np.ascontiguousarray(wqh[:, gsl]),
            "wql": np.ascontiguousarray(wql[:, gsl]),
            "wkh": np.ascontiguousarray(wkh[:, gsl]),
            "wkl": np.ascontiguousarray(wkl[:, gsl]),
            "wvh": np.ascontiguousarray(wvh[:, gsl]),
            "wvl": np.ascontiguousarray(wvl[:, gsl]),
            "wpt": np.ascontiguousarray(wpt[gsl, :]),
            "tri": tri,
        })
    return in_maps


def run_cores(x, Wq, Wk, Wv, Wp, bp, **spmd_kwargs):
    """Run the 8-core program; returns (y_full, BassKernelResults)."""
    nc = _get_program()
    in_maps = _make_in_maps(x, Wq, Wk, Wv, Wp)
    res = run_bass_kernel_spmd(nc, in_maps, list(range(N_CORES)), **spmd_kwargs)
    parts = [res.results[c]["y"] for c in range(N_CORES)]
    y = np.empty((B, T, E), np.float32)
    for b in range(B):
        acc = parts[4 * b].astype(np.float32)
        for hg in range(1, 4):
            acc = acc + parts[4 * b + hg].astype(np.float32)
        y[b] = acc
    y += np.asarray(bp, np.float32)[None, None, :]
    return y, res


def kernel(x, Wq, Wk, Wv, Wp, bp):
    y, _ = run_cores(x, Wq, Wk, Wv, Wp, bp)
    return y


def bench(x, Wq, Wk, Wv, Wp, bp, iters=12):
    """Time repeated on-device executions of the compiled program."""
    import time

    import jax
    import numpy as np_
    from jax.experimental.shard_map import shard_map
    from jax.sharding import Mesh, NamedSharding, PartitionSpec

    from concourse import bass2jax, mybir as mb

    nc = _get_program()
    in_maps = _make_in_maps(x, Wq, Wk, Wv, Wp)
    n_cores = N_CORES
    bass2jax.install_neuronx_cc_hook()

    partition_name = nc.partition_id_tensor.name if nc.partition_id_tensor else None
    in_names, out_names, out_avals, zero_outs = [], [], [], []
    for alloc in nc.m.functions[0].allocations:
        if not isinstance(alloc, mb.MemoryLocationSet):
            continue
        name = alloc.memorylocations[0].name
        if alloc.kind == "ExternalInput":
            if name != partition_name:
                in_names.append(name)
        elif alloc.kind == "ExternalOutput":
            out_names.append(name)
            shape = tuple(alloc.tensor_shape)
            dtype = mb.dt.np(alloc.dtype)
            out_avals.append(jax.core.ShapedArray(shape, dtype))
            zero_outs.append(np_.zeros(shape, dtype))
    n_params = len(in_names)
    all_in_names = in_names + out_names
    if partition_name is not None:
        all_in_names = all_in_names + [partition_name]

    def _body(*args):
        operands = list(args)
        if partition_name is not None:
            operands.append(bass2jax.partition_id_tensor())
        outs = bass2jax._bass_exec_p.bind(
            *operands,
            out_avals=tuple(out_avals),
            in_names=tuple(all_in_names),
            out_names=tuple(out_names),
            lowering_input_output_aliases=(),
            sim_require_finite=True,
            sim_require_nnan=True,
            nc=nc,
        )
        return tuple(outs)

    devices = jax.devices()[:n_cores]
    mesh = Mesh(np_.asarray(devices), ("core",))
    donate = tuple(range(n_params, n_params + len(out_names)))
    sharded = jax.jit(
        shard_map(_body, mesh=mesh,
                  in_specs=(PartitionSpec("core"),) * (n_params + len(out_names)),
                  out_specs=(PartitionSpec("core"),) * len(out_names),
                  check_rep=False),
        donate_argnums=donate, keep_unused=True,
    )
    sh = NamedSharding(mesh, PartitionSpec("core"))
    concat_in = [
        jax.device_put(
            np_.concatenate([np_.asarray(in_maps[c][nm]) for c in range(n_cores)], axis=0), sh)
        for nm in in_names
    ]
    zeros_np = [np_.zeros((n_cores * z.shape[0], *z.shape[1:]), z.dtype) for z in zero_outs]

    times = []
    out_arrs = None
    for it in range(iters):
        dz = [jax.device_put(z, sh) for z in zeros_np]
        jax.block_until_ready(dz)
        t0 = time.perf_counter()
        out_arrs = sharded(*concat_in, *dz)
        jax.block_until_ready(out_arrs)
        times.append(time.perf_counter() - t0)

    parts = [
        np_.asarray(out_arrs[i]).reshape(n_cores, *out_avals[i].shape)
        for i, nm in enumerate(out_names)
    ]
    yi = out_names.index("y")
    y = np_.empty((B, T, E), np_.float32)
    for b in range(B):
        acc = parts[yi][4 * b].astype(np_.float32)
        for hg in range(1, 4):
            acc = acc + parts[yi][4 * b + hg].astype(np_.float32)
        y[b] = acc
    y += np_.asarray(bp, np_.float32)[None, None, :]
    return y, times


# revision 40
# speedup vs baseline: 1.2263x; 1.0271x over previous
"""Multi-head causal attention (B=2, T=2048, E=1024, H=16, D=64) on 8 trn2 cores.

Sharding: core c -> batch b = c // 4, head-group hg = c % 4 (4 heads each).
Per-core: QKV projections for its 4 heads, causal flash attention in
transposed-score layout (S^T[k,q]; softmax denominator folded into a
ones-augmented V matmul), row-parallel output projection producing a partial
[T, E] output. Host sums the 4 partials per batch and adds the bias.

v3: all attention matmuls in fp8 DoubleRow mode (0.5 cyc/col):
 - S = K^T.Q: lhsT ktiles = (zeros-chunk, K-chunk) within one kt tile
   (step-sliced AP), rhs = Q broadcast stride-0 over the ktile dim.
   Q,K quantized to fp8e4m3 (single term x_hi*w_hi projections).
 - P.V: lhsT ktiles = (V_hi, V_lo) fp8 split (exact to ~0.03%), rhs = P
   (exp output written directly as fp8) broadcast stride-0.
 - Q/K projections 1-term (fp8 rounding dominates anyway); V stays 3-term
   hi/lo since V error enters the output linearly (small-neff rows).
Causal masking stays post-exp via tri-mask multiplies, moved to gpsimd.
Output projection stays bf16. qk drain copies moved to DVE (ACT is the
bottleneck: exp cols ~58us + per-instr overhead).
"""
import collections
import sys
from contextlib import ExitStack

sys.path.insert(0, "/opt/trn_rl_repo")

import ml_dtypes
import numpy as np

import concourse.bass as bass
import concourse.tile as tile
from concourse import bacc, mybir
from concourse.bass_utils import run_bass_kernel_spmd

F32 = mybir.dt.float32
BF16 = mybir.dt.bfloat16
FP8 = mybir.dt.float8e4
DR = mybir.MatmulPerfMode.DoubleRow
EXP = mybir.ActivationFunctionType.Exp
SUB = mybir.AluOpType.subtract
MUL = mybir.AluOpType.mult

WSCALE = 32.0           # host prescale on Wq/Wk/Wv for fp8 range; q,k,v come
                        # out x32, folded into the exp scale and into Wp

B, T, E, H = 2, 2048, 1024, 16
D = E // H              # 64
N_CORES = 8
GH = 4                  # heads per core
GE = GH * D             # 256 per-core projection width
SCALE = float(D) ** -0.5

TCH = 512               # projection t-chunk
NTCH = T // TCH         # 4
KC = 8                  # contraction chunks of 128 over E
KC2 = KC // 2
QB = 512                # attention q-block
NQB = T // QB           # 4
KB = 128                # attention k-block
NKB = T // KB           # 16
VSLOT = 80              # v8 per-head slot width (64 data + ones col + pad, %16)

PE_NS = 1e9 / 2.4e9     # per moving-free column (bf16)
ACT_NS = 1e9 / 1.2e9    # per free column
EXP_OVH = 217.0         # ACT per-instruction overhead (access + decode)

DEFAULT_OPTS = dict(
    s_bufs=2,
    o_bufs=2,
    pv_bufs=2,
    p_bufs=6,
    x_bufs=4,
    l_bufs=6,
    on_bufs=8,
    yst_bufs=4,
    norm_splits=1,       # normalize split count (qb < last)
    norm_splits_last=4,  # normalize split count for the last q-block
    sem_lat=400.0,       # pacing fudge: SS-end -> exp-start latency
    ret_lat=1000.0,      # pacing fudge: exp-end -> O-start latency
    lead=0.0,            # pacing margin (ns)
    end_fill=1200.0,     # filler ns pulled at each stream end (norm window)
    copy_cd=900.0,       # ns between a proj drain copy and the next slot alloc
    y_defer=1,           # 1: Y(qb) paced into phase qb+1; 3: all saved for last phase
    qk_copy_eng="vector",
    qk_terms=3,          # x*W terms for Q/K proj (1=hh only; 3=hh,lh,hl)
    tri_eng="vector",    # engine for the post-exp causal masking multiplies
    y_tail_split=True,   # alternate last-phase y drains between DVE and ACT
    early_qp0=True,      # drain next phase's qp0 at pair-1 start (DVE queue
                         # order: its copy lands before the phase-end bursts)
    sel_drain=True,      # drain() defers y units instead of emitting them
    cd_hist=1,           # which drain-copy (1=last) gates the next slot alloc
    early_act_copies=2,  # tch < this: qk/v drain copies go to ACT (idle early)
    warmup=80,           # junk DR matmuls at t=0 to climb the PE p-state ramp
    qk1_scalar=False,    # pair-1 q/k drain copies on ACT (idle at pair turn)
    y_si=3,              # y fillers allowed in streams si < y_si
)


def build_program(opts=None):
    o = dict(DEFAULT_OPTS)
    if opts:
        o.update(opts)
    nc = bacc.Bacc("TRN2", target_bir_lowering=False, debug=False, num_devices=N_CORES)

    xh_d = nc.dram_tensor("xh", [E, T], FP8, kind="ExternalInput").ap()
    xl_d = nc.dram_tensor("xl", [E, T], FP8, kind="ExternalInput").ap()
    wqh_d = nc.dram_tensor("wqh", [E, GE], FP8, kind="ExternalInput").ap()
    wql_d = nc.dram_tensor("wql", [E, GE], FP8, kind="ExternalInput").ap()
    wkh_d = nc.dram_tensor("wkh", [E, GE], FP8, kind="ExternalInput").ap()
    wkl_d = nc.dram_tensor("wkl", [E, GE], FP8, kind="ExternalInput").ap()
    wvh_d = nc.dram_tensor("wvh", [E, GE], FP8, kind="ExternalInput").ap()
    wvl_d = nc.dram_tensor("wvl", [E, GE], FP8, kind="ExternalInput").ap()
    wpt_d = nc.dram_tensor("wpt", [GE, E], BF16, kind="ExternalInput").ap()
    tri_d = nc.dram_tensor("tri", [KB, KB], FP8, kind="ExternalInput").ap()
    y_d = nc.dram_tensor("y", [T, E], BF16, kind="ExternalOutput").ap()

    with tile.TileContext(nc) as tc:
        with tc.tile_pool(name="weights", bufs=1) as wpool, \
             tc.tile_pool(name="qk", bufs=1) as qkpool, \
             tc.tile_pool(name="vsb", bufs=1) as vpool, \
             tc.tile_pool(name="xin", bufs=o["x_bufs"]) as xpool, \
             tc.tile_pool(name="ptile", bufs=o["p_bufs"]) as ppool, \
             tc.tile_pool(name="lbc", bufs=o["l_bufs"]) as lpool, \
             tc.tile_pool(name="onorm", bufs=o["on_bufs"]) as onpool, \
             tc.tile_pool(name="ystage", bufs=o["yst_bufs"]) as ypool, \
             tc.tile_pool(name="s_ps", bufs=o["s_bufs"], space="PSUM") as s_ps, \
             tc.tile_pool(name="pv_ps", bufs=o["pv_bufs"], space="PSUM") as pv_ps, \
             tc.tile_pool(name="o_ps", bufs=o["o_bufs"], space="PSUM") as o_ps:
            qk_ps = v_ps = pv_ps

            nqk = 2 if o["qk_terms"] > 1 else 1
            wq_sb = [wpool.tile([128, KC2, 2, GE], FP8, name=f"wq{i}") for i in range(nqk)]
            wk_sb = [wpool.tile([128, KC2, 2, GE], FP8, name=f"wk{i}") for i in range(nqk)]
            wv_sb = [wpool.tile([128, KC2, 2, GE], FP8, name=f"wv{i}") for i in range(2)]
            wp_sb = wpool.tile([128, 2, E], BF16)
            tri_sb = wpool.tile([KB, KB], FP8)

            # Q^T per pair: [p=(h%2,d), pair, t] fp8
            qt_sb = qkpool.tile([128, 2, T], FP8)
            # K^T chunked: chunks 0 and 33 = zeros (DR ktile partner / warmup
            # operands), chunks 1+pair*16+j = K^T[:, j*128:(j+1)*128]
            kt_sb = qkpool.tile([128, 2 * NKB + 2, KB], FP8)
            # V: [p=key%128, tblock, hi/lo, head*80 + (d | ones at 64 | pad)]
            v_sb = vpool.tile([128, NKB, 2, GH * VSLOT], FP8)

            xts = [None] * NTCH  # per-tch ([hi, lo]) [128, KC2, 2, TCH] tiles

            def dr(ap3):
                # dram [rows, n] -> DoubleRow sbuf layout [p, c2, i, n]
                return ap3.rearrange("(c i p) n -> p c i n", i=2, p=128)

            def w_dma(w_sb_t, w_d):
                nc.sync.dma_start(out=w_sb_t[:], in_=dr(w_d))

            def emit_x_dma(tch):
                xts[tch] = [xpool.tile([128, KC2, 2, TCH], FP8, tag="xt",
                                       name=f"x{tch}_{hl}") for hl in range(2)]
                xsrc = [x_d[:, tch * TCH:(tch + 1) * TCH] for x_d in (xh_d, xl_d)]
                if tch == 0:
                    # prologue: HWDGE serializes DMA issue at ~650ns each and
                    # transfers serialize on the DMA pipe, so order by first
                    # use and load only the pair-0 weight columns up front:
                    # the pair-0 Q/K projection chain gates the first exp.
                    def w_half(w_sb_t, w_d, pair):
                        sl = slice(pair * 128, (pair + 1) * 128)
                        nc.sync.dma_start(out=w_sb_t[:, :, :, sl],
                                          in_=dr(w_d)[:, :, :, sl])
                    nc.sync.dma_start(out=xts[0][0][:], in_=dr(xsrc[0]))
                    w_half(wq_sb[0], wqh_d, 0)
                    if nqk > 1:
                        nc.sync.dma_start(out=xts[0][1][:], in_=dr(xsrc[1]))
                        w_half(wq_sb[1], wql_d, 0)
                    w_half(wk_sb[0], wkh_d, 0)
                    if nqk > 1:
                        w_half(wk_sb[1], wkl_d, 0)
                    w_dma(wv_sb[0], wvh_d)
                    w_dma(wv_sb[1], wvl_d)
                    if nqk == 1:
                        nc.sync.dma_start(out=xts[0][1][:], in_=dr(xsrc[1]))
                    w_half(wq_sb[0], wqh_d, 1)
                    if nqk > 1:
                        w_half(wq_sb[1], wql_d, 1)
                    w_half(wk_sb[0], wkh_d, 1)
                    if nqk > 1:
                        w_half(wk_sb[1], wkl_d, 1)
                    nc.sync.dma_start(out=tri_sb[:], in_=tri_d)
                    emit_x_dma(1)
                    nc.sync.dma_start(
                        out=wp_sb[:], in_=wpt_d.rearrange("(c p) n -> p c n", p=128))
                else:
                    for hl in range(2):
                        nc.sync.dma_start(out=xts[tch][hl][:], in_=dr(xsrc[hl]))

            # ---- pacing state ------------------------------------------------
            clk = {"pe": 0.0, "act": 0.0, "allow_y": False}
            copy_hist = collections.deque([-1e9] * 8, maxlen=8)
            fillers = collections.deque()    # proj units (tag, pe_ns, fn, allocs)
            fillers_y = collections.deque()  # y units: only emitted mid-stream

            def mm(pe_ns):
                clk["pe"] += pe_ns

            def emit_from(q):
                tag, pe_ns, fn, _alloc = q.popleft()
                marks = fn() or ()
                clk["pe"] += pe_ns
                if "copy" in marks:
                    copy_hist.append(clk["pe"])

            def emit_one():
                emit_from(fillers)

            def cd_blocked():
                # hold back a unit that re-allocates a shared proj psum slot
                # until the drain copy cd_hist groups back had time to land
                return clk["pe"] < copy_hist[-o["cd_hist"]] + o["copy_cd"]

            def pace(target):
                # proj fillers first; y units only mid-stream (a y matmul
                # stalls the in-order PE queue on its DVE drain copy, so they
                # must never sit ahead of a phase's first S matmuls)
                while clk["pe"] < target - o["lead"]:
                    if fillers and not (fillers[0][3] and cd_blocked()):
                        emit_from(fillers)
                    elif fillers_y and clk["allow_y"]:
                        emit_from(fillers_y)
                    else:
                        break

            def drain(tag_pred):
                while any(tag_pred(t) for t, _, _, _ in fillers):
                    emit_one()

            def qk_drain_copy(dst, src, tch=99, pair=0):
                if (o["qk_copy_eng"] == "scalar" or tch < o["early_act_copies"]
                        or (pair == 1 and o["qk1_scalar"])):
                    nc.scalar.copy(out=dst, in_=src)
                else:
                    nc.vector.tensor_copy(out=dst, in_=src)

            # ---- projection units -------------------------------------------
            TERMS = ((0, 0), (1, 0), (0, 1))  # (w hi/lo, x hi/lo): hh, lh, hl

            QK_TERMS = TERMS[:o["qk_terms"]]

            def qk_mms(ph, w_sb, tch, pair, c2):
                for ti, (wl, xl) in enumerate(QK_TERMS):
                    nc.tensor.matmul(
                        ph[:],
                        w_sb[wl][:, c2, :, pair * 128:(pair + 1) * 128],
                        xts[tch][xl][:, c2, :, :],
                        start=(c2 == 0 and ti == 0),
                        stop=(c2 == KC2 - 1 and ti == len(QK_TERMS) - 1),
                        perf_mode=DR)

            QKC = (TCH // 2) * PE_NS * o["qk_terms"]  # pe-ns per qk unit

            def q_units(tch):
                qp_h = {}

                def q_u(pair, c2):
                    def fn():
                        if c2 == 0:
                            qp_h[pair] = qk_ps.tile([128, TCH], F32, tag="pv",
                                                    name=f"q_{tch}_{pair}")
                        qk_mms(qp_h[pair], wq_sb, tch, pair, c2)
                        if c2 == KC2 - 1:
                            qk_drain_copy(
                                qt_sb[:, pair, tch * TCH:(tch + 1) * TCH],
                                qp_h[pair][:], tch, pair)
                            return ("copy",)
                    return fn
                return [(f"qp{pair}", QKC, q_u(pair, c2), c2 == 0)
                        for pair in range(2) for c2 in range(KC2)]

            def kv_units(tch):
                kp_h = {}

                def k_u(pair, c2):
                    def fn():
                        if c2 == 0:
                            kp_h[pair] = qk_ps.tile([128, TCH], F32, tag="pv",
                                                    name=f"k_{tch}_{pair}")
                        qk_mms(kp_h[pair], wk_sb, tch, pair, c2)
                        if c2 == KC2 - 1:
                            ch = 1 + pair * NKB + tch * (TCH // KB)
                            qk_drain_copy(
                                kt_sb[:, ch:ch + TCH // KB, :],
                                kp_h[pair].rearrange("p (c n) -> p c n", n=KB), tch, pair)
                            return ("copy",)
                    return fn

                units = [(f"kp{pair}", QKC, k_u(pair, c2), c2 == 0)
                         for pair in range(2) for c2 in range(KC2)]
                vp_h = {}

                def v_u(tsub, half):
                    def fn():
                        if half == 0:
                            vp_h[tsub] = v_ps.tile([128, GE], F32, tag="pv",
                                                   name=f"vp{tch}_{tsub}")
                        for c2 in range(2 * half, 2 * half + 2):
                            for ti, (wl, xl) in enumerate(TERMS):
                                nc.tensor.matmul(
                                    vp_h[tsub][:],
                                    xts[tch][xl][:, c2, :, tsub * KB:(tsub + 1) * KB],
                                    wv_sb[wl][:, c2, :, :],
                                    start=(c2 == 0 and ti == 0),
                                    stop=(c2 == KC2 - 1 and ti == len(TERMS) - 1),
                                    perf_mode=DR)
                        if half == 1:
                            tb = tch * (TCH // KB) + tsub
                            vsrc = vp_h[tsub].rearrange("p (h c) -> p h c", c=D)
                            vhi = v_sb.rearrange(
                                "p b i (h w) -> p b i h w", w=VSLOT)[:, tb, 0, :, 0:D]
                            vlo = v_sb.rearrange(
                                "p b i (h w) -> p b i h w", w=VSLOT)[:, tb, 1, :, 0:D]
                            if tch < o["early_act_copies"]:
                                nc.scalar.copy(out=vhi, in_=vsrc)
                            else:
                                nc.vector.tensor_copy(out=vhi, in_=vsrc)
                            nc.vector.tensor_tensor(out=vlo, in0=vsrc, in1=vhi, op=SUB)
                            return ("copy",)
                    return fn

                # shared proj psum slots: groups must stay contiguous
                k0 = [u for u in units if u[0] == "kp0"]
                k1 = [u for u in units if u[0] == "kp1"]
                vs = [("v", 3 * GE * PE_NS, v_u(t, half), half == 0)
                      for t in range(TCH // KB) for half in range(2)]
                return k0, vs, k1

            # ---- output-projection units ------------------------------------
            def y_units(qb, onorms):
                q0 = qb * QB
                units = []
                yt_h = {}
                late = qb >= NQB - 2  # runs in phase 3 / tail: pv pool is idle

                def y_unit(qt, nh):
                    def fn():
                        if nh == 0:
                            yt_h[qt] = ypool.tile([128, E], BF16, tag="yt", name=f"yt{qt}")
                        if late:
                            yp = pv_ps.tile([128, 512], F32, tag="pv", name="yp")
                        else:
                            yp = s_ps.tile([128, 512], F32, tag="s", name="yp")
                        for pair in range(2):
                            nc.tensor.matmul(yp[:],
                                             onorms[pair][:, qt * 128:(qt + 1) * 128],
                                             wp_sb[:, pair, nh * 512:(nh + 1) * 512],
                                             start=(pair == 0), stop=(pair == 1))
                        ysl = yt_h[qt][:, nh * 512:(nh + 1) * 512]
                        if qb == NQB - 1 and o["y_tail_split"] and (qt + nh) % 2:
                            nc.scalar.copy(out=ysl, in_=yp[:])
                        else:
                            nc.vector.tensor_copy(out=ysl, in_=yp[:])
                        nc.sync.dma_start(
                            out=y_d[q0 + qt * 128:q0 + (qt + 1) * 128, nh * 512:(nh + 1) * 512],
                            in_=ysl)
                    return fn

                for qt in range(QB // 128):
                    for nh in range(2):
                        units.append(("y", 2 * 512 * PE_NS, y_unit(qt, nh), False))
                return units

            # ---- attention stream -------------------------------------------
            def vslot(hb, hl):
                # [128, 2, 65] hi/lo ktile view of head hb at key-block j
                def at(j):
                    base = v_sb.rearrange("p b i (h w) -> p b i h w", w=VSLOT)
                    return base[:, j, :, hb, 0:D + 1]
                return at

            def kdr(pair, j, h):
                # lhsT [64, 2, 128]: ktile 0 = zeros chunk, ktile 1 = K chunk
                c = 1 + pair * NKB + j
                return kt_sb[h * D:(h + 1) * D, 0:c + 1:c, :]

            def bcast2(ap):
                return ap.unsqueeze(1).broadcast_to([ap.shape[0], 2, ap.shape[1]])

            def bcast2p(ap):
                return ap.unsqueeze(1).broadcast_to([128, 2, ap.shape[1]])

            def normalize(o_p, onorm, h, splits=1):
                w = QB // splits
                for s in range(splits):
                    qs = slice(s * w, (s + 1) * w)
                    strip = lpool.tile([1, w], F32, tag="strip")
                    nc.vector.reciprocal(out=strip[:], in_=o_p[D:D + 1, qs])
                    lb = lpool.tile([D, w], F32, tag="lb")
                    nc.gpsimd.partition_broadcast(lb[:], strip[:])
                    nc.vector.tensor_mul(onorm[h * D:(h + 1) * D, qs], o_p[0:D, qs], lb[:])

            def stream(qb, pair, h, onorm, splits, prek=None, prev_v=None):
                q0 = qb * QB
                nk = (q0 + QB) // KB
                nfull = nk - 4
                bsl = slice(h * D, h * D + D)
                hb = pair * 2 + h
                vat = vslot(hb, 0)
                o_p = o_ps.tile([D + 1, QB], F32, tag="o")
                qrhs = qt_sb[bsl, pair, q0:q0 + QB]

                def grp(j2, diag):
                    r0 = (j2 - nfull) * KB if diag else 0
                    r1 = r0 + KB
                    w1 = QB - r1

                    def ss():
                        sp = s_ps.tile([128, 2 * QB], F32, tag="s", name="sp")
                        pt = ppool.tile([128, 2 * QB], FP8, tag="p", name="pt")
                        if diag:
                            nc.tensor.matmul(sp[:, r0:QB], kdr(pair, j2, h),
                                             bcast2(qrhs[:, r0:]),
                                             start=True, stop=True, perf_mode=DR)
                            nc.tensor.matmul(sp[:, QB:QB + w1], kdr(pair, j2 + 1, h),
                                             bcast2(qrhs[:, r1:]),
                                             start=True, stop=True, perf_mode=DR)
                            mm((QB - r0 + w1) * PE_NS / 2)
                            nc.scalar.activation(out=pt[:, r0:QB + w1], in_=sp[:, r0:QB + w1],
                                                 func=EXP, scale=SCALE / (WSCALE * WSCALE))
                            clk["act"] = max(clk["act"], clk["pe"] + o["sem_lat"]) \
                                + (QB - r0 + w1) * ACT_NS + EXP_OVH
                        else:
                            for jj in range(2):
                                j = j2 + jj
                                nc.tensor.matmul(sp[:, jj * QB:(jj + 1) * QB],
                                                 kdr(pair, j, h), bcast2(qrhs),
                                                 start=True, stop=True, perf_mode=DR)
                            mm(2 * QB * PE_NS / 2)
                            nc.scalar.activation(out=pt[:], in_=sp[:], func=EXP,
                                                 scale=SCALE / (WSCALE * WSCALE))
                            clk["act"] = max(clk["act"], clk["pe"] + o["sem_lat"]) \
                                + 2 * QB * ACT_NS + EXP_OVH
                        return pt

                    def oo(pt):
                        if diag:
                            tri_e = nc.gpsimd if o["tri_eng"] == "pool" else nc.vector
                            tri_e.tensor_tensor(out=pt[:, r0:r0 + KB],
                                                in0=pt[:, r0:r0 + KB], in1=tri_sb[:], op=MUL)
                            tri_e.tensor_tensor(out=pt[:, QB:QB + KB],
                                                in0=pt[:, QB:QB + KB], in1=tri_sb[:], op=MUL)
                            nc.tensor.matmul(o_p[:, r0:QB], vat(j2),
                                             bcast2p(pt[:, r0:QB]),
                                             start=(j2 == 0), stop=False, perf_mode=DR)
                            nc.tensor.matmul(o_p[:, r1:QB], vat(j2 + 1),
                                             bcast2p(pt[:, QB:QB + w1]),
                                             start=False, stop=(j2 + 1 == nk - 1),
                                             perf_mode=DR)
                            mm((QB - r0 + w1) * PE_NS / 2)
                        else:
                            for jj in range(2):
                                j = j2 + jj
                                nc.tensor.matmul(o_p[:], vat(j),
                                                 bcast2p(pt[:, jj * QB:(jj + 1) * QB]),
                                                 start=(j == 0), stop=False, perf_mode=DR)
                            mm(2 * QB * PE_NS / 2)
                    return ss, oo

                groups = [grp(j2, False) for j2 in range(0, nfull, 2)] \
                    + [grp(j2, True) for j2 in range(nfull, nk, 2)]
                # software-pipelined: SS/exp of group g+1 runs before OO of g
                # so the next S matmuls (plus fillers) cover the exp latency
                prev = None
                ndiag = len(groups) - 2  # first group whose SS reads this tch's K
                for gi, (ss, oo) in enumerate(groups):
                    if gi == ndiag and prek is not None:
                        prek()
                    pt = ss()
                    pace(clk["act"] + o["ret_lat"])
                    if prev is not None:
                        if prev[2] == ndiag and prev_v is not None:
                            prev_v()
                        prev[1](prev[0])
                    prev = (pt, oo, gi)
                if prev[2] == ndiag and prev_v is not None:
                    prev_v()
                prev[1](prev[0])
                normalize(o_p, onorm, h, splits)
                pace(clk["pe"] + o["end_fill"])

            # ---- main schedule ----------------------------------------------
            nc.vector.memset(kt_sb[:, 0, :], 0.0)
            nc.vector.memset(kt_sb[:, 2 * NKB + 1, :], 0.0)
            if o["warmup"]:
                # climb the PE p-state ramp on junk zero matmuls while the
                # prologue DMAs land; results are discarded
                wu_ps = s_ps.tile([128, 2 * QB], F32, tag="s", name="wu")
                wu_lhs = kt_sb[0:64, 0::(2 * NKB + 1), :]
                wu_rhs = kt_sb[0:64, 0, :].unsqueeze(1).broadcast_to([64, 2, KB])
                for _ in range(o["warmup"]):
                    nc.tensor.matmul(wu_ps[:, 0:KB], wu_lhs, wu_rhs,
                                     start=True, stop=True, perf_mode=DR)
            emit_x_dma(0)
            vv = v_sb.rearrange("p b i (h w) -> p b i h w", w=VSLOT)
            nc.vector.memset(vv[:, :, 0, :, D:D + 1], 1.0)
            nc.vector.memset(vv[:, :, 1, :, D:D + 1], 0.0)

            def queue_proj(tch):
                qs = q_units(tch)
                q0 = [u for u in qs if u[0] == "qp0"]
                q1 = [u for u in qs if u[0] == "qp1"]
                k0, vs, k1 = kv_units(tch)
                for t, c, f, a in q0 + k0 + vs + q1 + k1:
                    fillers.append((f"{t}@{tch}", c, f, a))

            pending_y = []
            for qb in range(NQB):
                if qb == 0:
                    queue_proj(0)
                if 0 < qb < NQB - 1:
                    emit_x_dma(qb + 1)
                # this phase's Q (and for streams' diagonals, K/V) must be
                # emitted before the attention that reads it
                drain(lambda t: t == f"qp0@{qb}")
                if qb < NQB - 1:
                    queue_proj(qb + 1)
                onorms = [onpool.tile([128, QB], BF16, tag="on", name=f"on{qb}_{i}") for i in range(2)]
                splits = o["norm_splits_last"] if qb == NQB - 1 else o["norm_splits"]
                for si, (pair, h) in enumerate(((0, 0), (0, 1), (1, 0), (1, 1))):
                    clk["allow_y"] = si < o["y_si"] or qb == NQB - 1
                    if pair == 1 and h == 0:
                        drain(lambda t: t == f"qp1@{qb}")
                        if o["early_qp0"] and qb < NQB - 1:
                            drain(lambda t: t == f"qp0@{qb + 1}")
                    prek = (lambda p=pair: drain(
                        lambda t: t == f"kp{p}@{qb}")) if h == 0 else None
                    prev_v = (lambda: drain(
                        lambda t: t == f"v@{qb}")) if (pair == 0 and h == 0) else None
                    stream(qb, pair, h, onorms[pair], splits, prek=prek, prev_v=prev_v)
                    if si == 0 and pending_y and (
                            qb == NQB - 1 or o["y_defer"] == 1):
                        fillers_y.extend(pending_y)
                        pending_y = []
                pending_y = pending_y + y_units(qb, onorms)
            clk["allow_y"] = True
            while fillers or fillers_y:
                emit_from(fillers if fillers else fillers_y)
            for _, _, fn, _a in pending_y:
                fn()

    nc.compile()
    return nc


_NC = {}


def _get_program(opts=None):
    key = tuple(sorted((opts or {}).items()))
    if key not in _NC:
        _NC[key] = build_program(opts)
    return _NC[key]


def _make_in_maps(x, Wq, Wk, Wv, Wp):
    bf = ml_dtypes.bfloat16
    f8 = ml_dtypes.float8_e4m3
    x32 = np.asarray(x, np.float32)
    xh = x32.astype(f8)
    xl = (x32 - xh.astype(np.float32)).astype(f8)

    def wsplit(W):
        wt = np.asarray(W, np.float32).T * WSCALE
        hi = wt.astype(f8)
        lo = (wt - hi.astype(np.float32)).astype(f8)
        return hi, lo

    wqh, wql = wsplit(Wq)
    wkh, wkl = wsplit(Wk)
    wvh, wvl = wsplit(Wv)
    wpt = np.ascontiguousarray((np.asarray(Wp, np.float32).T / WSCALE).astype(bf))
    tri = (np.arange(KB)[:, None] <= np.arange(KB)[None, :]).astype(f8)
    in_maps = []
    for c in range(N_CORES):
        b, hg = c // 4, c % 4
        gsl = slice(hg * GE, (hg + 1) * GE)
        in_maps.append({
            "xh": np.ascontiguousarray(xh[b].T),
            "xl": np.ascontiguousarray(xl[b].T),
            "wqh": np.ascontiguousarray(wqh[:, gsl]),
            "wql": np.ascontiguousarray(wql[:, gsl]),
            "wkh": np.ascontiguousarray(wkh[:, gsl]),
            "wkl": np.ascontiguousarray(wkl[:, gsl]),
            "wvh": np.ascontiguousarray(wvh[:, gsl]),
            "wvl": np.ascontiguousarray(wvl[:, gsl]),
            "wpt": np.ascontiguousarray(wpt[gsl, :]),
            "tri": tri,
        })
    return in_maps


def run_cores(x, Wq, Wk, Wv, Wp, bp, **spmd_kwargs):
    """Run the 8-core program; returns (y_full, BassKernelResults)."""
    nc = _get_program()
    in_maps = _make_in_maps(x, Wq, Wk, Wv, Wp)
    res = run_bass_kernel_spmd(nc, in_maps, list(range(N_CORES)), **spmd_kwargs)
    parts = [res.results[c]["y"] for c in range(N_CORES)]
    y = np.empty((B, T, E), np.float32)
    for b in range(B):
        acc = parts[4 * b].astype(np.float32)
        for hg in range(1, 4):
            acc = acc + parts[4 * b + hg].astype(np.float32)
        y[b] = acc
    y += np.asarray(bp, np.float32)[None, None, :]
    return y, res


def kernel(x, Wq, Wk, Wv, Wp, bp):
    y, _ = run_cores(x, Wq, Wk, Wv, Wp, bp)
    return y


def bench(x, Wq, Wk, Wv, Wp, bp, iters=12):
    """Time repeated on-device executions of the compiled program."""
    import time

    import jax
    import numpy as np_
    from jax.experimental.shard_map import shard_map
    from jax.sharding import Mesh, NamedSharding, PartitionSpec

    from concourse import bass2jax, mybir as mb

    nc = _get_program()
    in_maps = _make_in_maps(x, Wq, Wk, Wv, Wp)
    n_cores = N_CORES
    bass2jax.install_neuronx_cc_hook()

    partition_name = nc.partition_id_tensor.name if nc.partition_id_tensor else None
    in_names, out_names, out_avals, zero_outs = [], [], [], []
    for alloc in nc.m.functions[0].allocations:
        if not isinstance(alloc, mb.MemoryLocationSet):
            continue
        name = alloc.memorylocations[0].name
        if alloc.kind == "ExternalInput":
            if name != partition_name:
                in_names.append(name)
        elif alloc.kind == "ExternalOutput":
            out_names.append(name)
            shape = tuple(alloc.tensor_shape)
            dtype = mb.dt.np(alloc.dtype)
            out_avals.append(jax.core.ShapedArray(shape, dtype))
            zero_outs.append(np_.zeros(shape, dtype))
    n_params = len(in_names)
    all_in_names = in_names + out_names
    if partition_name is not None:
        all_in_names = all_in_names + [partition_name]

    def _body(*args):
        operands = list(args)
        if partition_name is not None:
            operands.append(bass2jax.partition_id_tensor())
        outs = bass2jax._bass_exec_p.bind(
            *operands,
            out_avals=tuple(out_avals),
            in_names=tuple(all_in_names),
            out_names=tuple(out_names),
            lowering_input_output_aliases=(),
            sim_require_finite=True,
            sim_require_nnan=True,
            nc=nc,
        )
        return tuple(outs)

    devices = jax.devices()[:n_cores]
    mesh = Mesh(np_.asarray(devices), ("core",))
    donate = tuple(range(n_params, n_params + len(out_names)))
    sharded = jax.jit(
        shard_map(_body, mesh=mesh,
                  in_specs=(PartitionSpec("core"),) * (n_params + len(out_names)),
                  out_specs=(PartitionSpec("core"),) * len(out_names),
                  check_rep=False),
        donate_argnums=donate, keep_unused=True,
    )
    sh = NamedSharding(mesh, PartitionSpec("core"))
    concat_in = [
        jax.device_put(
            np_.concatenate([np_.asarray(in_maps[c][nm]) for c in range(n_cores)], axis=0), sh)
        for nm in in_names
    ]
    zeros_np = [np_.zeros((n_cores * z.shape[0], *z.shape[1:]), z.dtype) for z in zero_outs]

    times = []
    out_arrs = None
    for it in range(iters):
        dz = [jax.device_put(z, sh) for z in zeros_np]
        jax.block_until_ready(dz)
        t0 = time.perf_counter()
        out_arrs = sharded(*concat_in, *dz)
        jax.block_until_ready(out_arrs)
        times.append(time.perf_counter() - t0)

    parts = [
        np_.asarray(out_arrs[i]).reshape(n_cores, *out_avals[i].shape)
        for i, nm in enumerate(out_names)
    ]
    yi = out_names.index("y")
    y = np_.empty((B, T, E), np_.float32)
    for b in range(B):
        acc = parts[yi][4 * b].astype(np_.float32)
        for hg in range(1, 4):
            acc = acc + parts[yi][4 * b + hg].astype(np_.float32)
        y[b] = acc
    y += np_.asarray(bp, np_.float32)[None, None, :]
    return y, times
